# revision 12
# baseline (speedup 1.0000x reference)
"""Trainium2 Bass kernel for nn_CrossAttentionModule (head-collapsed cross attention).

Math (reference):
    Q = x @ Wq.T ; K = y @ Wk.T ; V = y @ Wv.T          (torch Linear convention)
    energy[n,q,k] = sum_{h,d} Q[n,q,h,d] K[n,k,h,d]     (heads summed!)
    att = softmax(energy / sqrt(512), axis=k)
    out = x + (att @ V) @ Wo.T + bo

Because heads are summed, energy = x @ (Wq.T @ Wk) @ y.T and the output
projection folds into V:  (att @ V) @ Wo.T = att @ (y @ (Wo @ Wv).T).
So we precompute on host (512x512, trivial):
    A    = Wq.T @ Wk        -> energy = (x @ A) @ y.T
    WvoT = Wv.T @ Wo.T      -> Vp = y @ WvoT ; att_out = att @ Vp
Device (per core, data-parallel over the N=8 batch):
    tT = A.T @ xT           [e2, q]   fp8 pair-packed
    Vp = y @ WvoT           [k, f]    fp8
    S^T tiles  = yT.T @ tT  [k, q]    fp32 psum   (k on partitions)
    P = exp(S^T * 1/sqrt(512) - C)    fp8
    att_psum  += P.T @ Vp   [q, f]    fp32 psum   (accumulated over k pairs)
    den_psum  += P.T @ ones [q, 1]    fp32 psum
    out = att_psum * (1/den)          bf16 -> DRAM
Host adds the residual x + out + bo in fp32.
"""

import sys

sys.path.insert(0, "/opt/trn_rl_repo")

import ml_dtypes
import numpy as np

import bass_rust
import concourse.bass as bass
import concourse.mybir as mybir
import concourse.tile as tile
from concourse.bass_utils import run_bass_kernel_spmd
from concourse.vector_clock import ScopedClock

N_CORES = 8
E = 512  # embed dim
Q = 2048  # query length (per batch element)
K = 4096  # key/value length
P = 128  # partitions
ET = E // P  # 4 embed tiles
QB = 512  # q block width for S^T matmuls
NQB = Q // QB  # 4
QS = P  # q sub-block (att psum partition dim)
NQS = QB // QS  # 4
KT = K // P  # 32 k tiles
SCALE = float(1.0 / np.sqrt(np.float32(512.0)))

BF16 = mybir.dt.bfloat16
F32 = mybir.dt.float32
FP8E4 = mybir.dt.float8e4
BF16_NP = ml_dtypes.bfloat16
E4_NP = ml_dtypes.float8_e4m3


def _patched_drain_and_barrier(self, tick_clock, wait_clock):
    # The walrus build in this container caps sync-wait commands per CTRL
    # instruction below what Tile's tail drain emits; split the waits across
    # separate SP nops (same engine => same ordering semantics).
    nc = self.nc
    probe = nc.sync.nop(nofuse=True)
    wait_clock.add_sem_waits(probe.ins, ScopedClock({None: tick_clock.global_clock}))
    waits = list(probe.ins.sync_info.on_wait)
    probe.ins.sync_info = bass_rust.SyncInfo(on_wait=waits[:1], on_update=[])
    for wval in waits[1:]:
        n2 = nc.sync.nop(nofuse=True)
        n2.ins.sync_info = bass_rust.SyncInfo(on_wait=[wval], on_update=[])
    nc.sync.drain()
    nc.all_engine_barrier()
    popped = nc._tile_sem_poison_stack.pop()
    assert popped is self._sem_poison
    # Inline clear_and_free_semaphores, but spread the sem clears over all
    # engines (they serialize ~30ns each; ~250 sems on one engine is ~7us of
    # tail). dma_reset must stay on gpsimd. No trailing all_engine_barrier:
    # NEFF completion waits for every engine to halt anyway, so the next
    # execution still sees cleared semaphores.
    from concourse.bass import compact_to_ranges

    sems = list(self.sems.allocated().values())
    if sems:
        sem_nums = [s.num if hasattr(s, "num") else s for s in sems]
        engines = [nc.gpsimd, nc.vector, nc.scalar, nc.tensor, nc.sync]
        for sem_range in compact_to_ranges(sem_nums):
            assert nc._state.free_isdisjoint(sem_range)
            nc.gpsimd.dma_reset(sem_range)
            n = len(sem_range)
            n_eng = len(engines)
            step = (n + n_eng - 1) // n_eng
            for ei, lo in enumerate(range(0, n, step)):
                sub = range(sem_range.start + lo, sem_range.start + min(lo + step, n))
                engines[ei % n_eng].sem_clear(sub)
        nc._state.prepend_free_semaphores(sem_nums)
        for poison_set in nc._tile_sem_poison_stack:
            poison_set.update(sem_nums)


tile.TileContext._drain_and_barrier = _patched_drain_and_barrier

_MAX_WAITS = 1  # walrus merges Ldweights+Matmult waits into one struct capped at 2


def _split_sync_waits(nc, max_waits=_MAX_WAITS):
    # Hoist sem waits beyond the per-instruction cap onto same-engine NoOps
    # inserted right before the offender (same engine => same order semantics).
    # For Matmult preceded by its Ldweights, nops go before the Ldweights so
    # walrus can still fuse the pair (their waits are summed in the MM struct).
    n_nops = 0
    for f in nc.m.functions:
        for bb in f.blocks:
            new_insts = []
            changed = False
            for inst in bb.instructions:
                si = getattr(inst, "sync_info", None)
                waits = list(si.on_wait) if si is not None else []
                if len(waits) > max_waits:
                    head, rest = waits[:-max_waits], waits[-max_waits:]
                    pos = len(new_insts)
                    if (
                        isinstance(inst, mybir.InstMatmult)
                        and new_insts
                        and isinstance(new_insts[-1], mybir.InstLdweights)
                    ):
                        pos -= 1
                    nops = []
                    for i0 in range(0, len(head), max_waits):
                        nops.append(
                            mybir.InstNoOp(
                                name=f"{inst.name}-wsplit{i0}",
                                sync_info=mybir.SyncInfo(
                                    on_wait=head[i0 : i0 + max_waits], on_update=[]
                                ),
                                bass_nofuse=True,
                                engine=inst.engine,
                            )
                        )
                        n_nops += 1
                    new_insts[pos:pos] = nops
                    inst.sync_info = mybir.SyncInfo(
                        on_wait=rest, on_update=list(si.on_update)
                    )
                    changed = True
                new_insts.append(inst)
            if changed:
                bb.instructions = new_insts
    return n_nops


def _build_fp8():
    """fp8 DoubleRow variant: contraction dims pair-packed as [128, 2, n].

    Pair layout: virtual contraction row (pair, p, i) = index pair*256 + i*128 + p.
    lhsT and rhs use the same (p, i) mapping, so the DoubleRow pairing is
    consistent regardless of the hardware's internal interleave order.

    DRAM layouts chosen for large contiguous-per-partition DMA rows and
    fine-grained phase-1 startup:
      A8d  [128, 4, 2, 2, 128]   : e2-block minor (512B rows, 4 x 64KB)
      Wv8d [2, 128, 2, 512]      : pr major (1KB rows)
      x8d  [4, 2, 128, 2, 512]   : (qb, pr) tiles (1KB rows)
      y8d  [4, 2, 128, 2, 1024]  : (quarter, pr) tiles (2KB rows)
    """
    nc = bass.Bass()
    A8d = nc.dram_tensor("A8d", [P, 4, 2, 2, P], FP8E4, kind="ExternalInput")
    Wv8d = nc.dram_tensor("Wv8d", [P, 2, 2, E], FP8E4, kind="ExternalInput")
    x8d = nc.dram_tensor("x8d", [4, P, 2, 2, QB], FP8E4, kind="ExternalInput")
    # duplicate of x8d[0][..., 0:128], contiguous: phase 1's first column block
    # can start on a 64KB load instead of waiting for the full 256KB qb0 tile
    x0h = nc.dram_tensor("x0h", [P, 2, 2, P], FP8E4, kind="ExternalInput")
    y8d = nc.dram_tensor("y8d", [4, P, 2, 2, K // 4], FP8E4, kind="ExternalInput")
    out = nc.dram_tensor("out", [Q, E], BF16, kind="ExternalOutput")

    exp = mybir.ActivationFunctionType.Exp
    DR = mybir.MatmulPerfMode.DoubleRow
    KP = KT // 2  # 16 k-pair tiles
    YQ = K // 4  # 1024
    # exp shift: P' = exp(s/sqrt(512) - C) fits e4m3 (max logit ~8.1 -> P' <= 62);
    # the flushed tail (weights < 2^-9 of e^C) carries ~1e-3 of the softmax mass.
    C_SHIFT = 4.0

    with tile.TileContext(nc) as tc:
        with (
            tc.tile_pool(name="const", bufs=1) as cpool,
            tc.tile_pool(name="pwork", bufs=4) as wpool,
            tc.tile_pool(name="outp", bufs=1) as opool,
            tc.tile_pool(name="ps_mm", bufs=3, space="PSUM") as ps_mm,
            tc.tile_pool(name="ps_att", bufs=1, space="PSUM") as ps_att,
            tc.tile_pool(name="ps_den", bufs=1, space="PSUM") as ps_den,
        ):
            A8_sb = cpool.tile([P, 4, 2, 2, P], FP8E4, name="A8sb")
            Wv8_sb = cpool.tile([P, 2, 2, E], FP8E4, name="Wv8sb")
            x8_sb = [cpool.tile([P, 2, 2, QB], FP8E4, name=f"x8{qb}") for qb in range(4)]
            x0h_sb = cpool.tile([P, 2, 2, P], FP8E4, name="x0h")
            y8_sb = [cpool.tile([P, 2, 2, YQ], FP8E4, name=f"y8{b}") for b in range(4)]
            t8_sb = [cpool.tile([P, 2, Q], FP8E4, name=f"t8{i}") for i in range(2)]
            Vp8_sb = [cpool.tile([P, 2, E], FP8E4, name=f"Vp8{i}") for i in range(KP)]
            ones_sb = cpool.tile([P, 32], FP8E4, name="ones")
            nc.vector.memset(ones_sb[:], 1.0)
            bias_sb = cpool.tile([P, 1], F32, name="biasC")
            nc.vector.memset(bias_sb[:], -C_SHIFT)
            # rhs AP [128, 2, 1] with middle step 16 (DoubleRow needs step%16==0)
            ones_ap = ones_sb.rearrange("p (i c) -> p i c", c=16)[:, :, 0:1]

            def y8sl(pr, klo, khi):
                b = klo // YQ
                assert (khi - 1) // YQ == b
                return y8_sb[b][:, pr, :, klo - b * YQ : khi - b * YQ]

            # Input DMAs, phase-1's first needs lead: A8 block0 + x8 qb0.
            # x8 rides the sync queue only (the gpsimd software queue spins up
            # ~2us later and would stall phase 1's first psum).
            nc.sync.dma_start(A8_sb[:, 0:1], A8d[:, 0:1])
            nc.sync.dma_start(x0h_sb[:], x0h[:])
            nc.sync.dma_start(x8_sb[0][:], x8d[0, :])
            nc.scalar.dma_start(A8_sb[:, 1:4], A8d[:, 1:4])
            nc.sync.dma_start(x8_sb[1][:], x8d[1, :])
            nc.sync.dma_start(x8_sb[2][:], x8d[2, :])
            nc.scalar.dma_start(x8_sb[3][:], x8d[3, :])
            nc.scalar.dma_start(Wv8_sb[:], Wv8d[:])
            # y8 is large and only needed from phase 2 on; defer most of it
            # behind phase-1 progress so the xT/A loads get full DMA bandwidth
            y8_dmas = []
            for b in range(4):
                eng = nc.gpsimd if b % 2 == 0 else nc.scalar
                dma = eng.dma_start(y8_sb[b][:], y8d[b, :])
                if b >= 1:
                    y8_dmas.append(dma)

            # Phase 1 (fp8 DR): tT[e2, q] = sum_e A[e, e2] * x[q, e], cast to fp8
            # pairs. qb-major so x8 qb0 unblocks the first 4 psums.
            p1_mms = []
            for qb in range(Q // 512):
                for e2 in range(ET):
                    pt = ps_mm.tile([P, 512], F32, name="ps_s")
                    for pr in range(2):
                        if qb == 0:
                            # ramp: first 128 q columns come from the small
                            # head chunk, the rest from the full qb0 tile
                            mm = nc.tensor.matmul(
                                pt[:, 0:P],
                                A8_sb[:, e2, pr],
                                x0h_sb[:, pr],
                                start=(pr == 0),
                                stop=(pr == 1),
                                perf_mode=DR,
                            )
                            p1_mms.append(mm)
                            mm = nc.tensor.matmul(
                                pt[:, P:512],
                                A8_sb[:, e2, pr],
                                x8_sb[qb][:, pr, :, P:512],
                                start=(pr == 0),
                                stop=(pr == 1),
                                perf_mode=DR,
                            )
                            p1_mms.append(mm)
                        else:
                            mm = nc.tensor.matmul(
                                pt[:],
                                A8_sb[:, e2, pr],
                                x8_sb[qb][:, pr],
                                start=(pr == 0),
                                stop=(pr == 1),
                                perf_mode=DR,
                            )
                            p1_mms.append(mm)
                    if (qb * ET + e2) % 2 == 0:
                        nc.vector.tensor_copy(
                            t8_sb[e2 // 2][:, e2 % 2, qb * 512 : (qb + 1) * 512], pt[:]
                        )
                    else:
                        nc.scalar.copy(
                            t8_sb[e2 // 2][:, e2 % 2, qb * 512 : (qb + 1) * 512], pt[:]
                        )
            # release deferred y8 loads once phase 1 is underway
            for dma in y8_dmas:
                tile.add_dep_helper(
                    dma.ins, p1_mms[4].ins, sync=True, reason="defer y8 behind xT"
                )

            # Phase 2 (fp8 DR): Vp[k, f] = sum_e2 y[k, e2] WvoT[e2, f], pair-packed.
            # psum->sbuf copies alternate ACT/DVE so neither engine falls behind
            # the PE (each alone would be ~40% slower than the PE's cadence).
            for kt in range(KT):
                pv = ps_mm.tile([P, 512], F32, name="ps_s")
                for pr in range(2):
                    nc.tensor.matmul(
                        pv[:],
                        y8sl(pr, kt * P, (kt + 1) * P),
                        Wv8_sb[:, pr],
                        start=(pr == 0),
                        stop=(pr == 1),
                        perf_mode=DR,
                    )
                if kt % 2 == 0:
                    nc.scalar.copy(Vp8_sb[kt // 2][:, kt % 2, :], pv[:])
                else:
                    nc.vector.tensor_copy(Vp8_sb[kt // 2][:, kt % 2, :], pv[:])

            # Phase 3: attention, one flat (qb, kp) pipeline. S^T/exp for
            # global step gi is emitted before the att/den matmuls of step
            # gi-1, including across qb boundaries, so the PE never waits on
            # ACT (exp) or on the epilogue of the previous q block.
            GT = NQB * KP
            p8_tiles = {}
            att_state = {}
            for gi in range(GT + 1):
                if gi < GT:
                    qb_s, kp_s = divmod(gi, KP)
                    p8 = wpool.tile([P, 2, QB], FP8E4, name="p8")
                    p8_tiles[gi] = p8
                    for half in range(2):
                        kt = 2 * kp_s + half
                        st = ps_mm.tile([P, QB], F32, name="ps_s")
                        for pr in range(2):
                            nc.tensor.matmul(
                                st[:],
                                y8sl(pr, kt * P, (kt + 1) * P),
                                t8_sb[pr][:, :, qb_s * QB : (qb_s + 1) * QB],
                                start=(pr == 0),
                                stop=(pr == 1),
                                perf_mode=DR,
                            )
                        nc.scalar.activation(
                            p8[:, half, :], st[:], exp, bias=bias_sb[:], scale=SCALE
                        )
                if gi >= 1:
                    qb, kprev = divmod(gi - 1, KP)
                    if kprev == 0:
                        att_state[qb] = (
                            [ps_att.tile([P, E], F32, name=f"att{j}") for j in range(NQS)],
                            ps_den.tile([P, NQS], F32, name="den"),
                        )
                    att_ps, den_ps = att_state[qb]
                    last = kprev == KP - 1
                    p8p = p8_tiles.pop(gi - 1)
                    for j in range(NQS):
                        # den before att in the last round so the epilogue's
                        # single reciprocal can issue before the last att
                        den_mm = lambda: nc.tensor.matmul(
                            den_ps[:, j : j + 1],
                            p8p[:, :, j * QS : (j + 1) * QS],
                            ones_ap,
                            start=(kprev == 0),
                            stop=last,
                            perf_mode=DR,
                        )
                        if last:
                            den_mm()
                        nc.tensor.matmul(
                            att_ps[j][:],
                            p8p[:, :, j * QS : (j + 1) * QS],
                            Vp8_sb[kprev][:],
                            start=(kprev == 0),
                            stop=last,
                            perf_mode=DR,
                        )
                        if not last:
                            den_mm()
                    if last:
                        # Epilogue: one reciprocal, then per-j scale split
                        # DVE/ACT halves (both read PSUM at 1 elem/cycle);
                        # DMA each j on the (idle) sync queue. This overlaps
                        # the next q-block's S^T/att matmuls, freeing each
                        # att psum bank just before its reuse.
                        rec_sb = opool.tile([P, NQS], F32, name="rec")
                        nc.vector.reciprocal(rec_sb[:], den_ps[:])
                        for j in range(NQS):
                            o_sb = opool.tile([P, E], BF16, name=f"osb{j}")
                            if j % 2 == 0:
                                nc.vector.tensor_scalar_mul(
                                    o_sb[:], att_ps[j][:], rec_sb[:, j : j + 1]
                                )
                            else:
                                nc.scalar.mul(
                                    o_sb[:], att_ps[j][:], rec_sb[:, j : j + 1]
                                )
                            (nc.sync if j % 2 == 0 else nc.gpsimd).dma_start(
                                out[qb * QB + j * QS : qb * QB + (j + 1) * QS, :],
                                o_sb[:],
                            )

    _split_sync_waits(nc)
    return nc


_CACHED_NC = None


def _get_nc():
    global _CACHED_NC
    if _CACHED_NC is None:
        _CACHED_NC = _build_fp8()
    return _CACHED_NC


def _pair_pack(m):
    # [512, n] -> [2, 128, 2, n] with (pair, p, i) -> row pair*256 + i*128 + p
    n = m.shape[1]
    return np.ascontiguousarray(m.reshape(2, 2, P, n).transpose(0, 2, 1, 3))


def _prep_inputs(x, y, Wq, Wk, Wv, Wo):
    A8 = _pair_pack((Wq.T @ Wk).astype(E4_NP))  # [2pr, 128, 2, 512]
    WvoT8 = _pair_pack((Wv.T @ Wo.T).astype(E4_NP))
    # A8d [128, 4 e2blk, 2pr, 2i, 128]
    A8d = np.ascontiguousarray(
        A8.reshape(2, P, 2, 4, P).transpose(1, 3, 0, 2, 4)
    )
    # Wv8d [128, 2pr, 2i, 512]
    Wv8d = np.ascontiguousarray(WvoT8.transpose(1, 0, 2, 3))
    x8 = np.stack([_pair_pack(x[n].T.astype(E4_NP)) for n in range(N_CORES)])
    y8 = np.stack([_pair_pack(y[n].T.astype(E4_NP)) for n in range(N_CORES)])
    # x8 [N, 2pr, 128, 2, 2048] -> [N, 4qb, 128, 2pr, 2i, 512]
    x8 = np.ascontiguousarray(
        x8.reshape(N_CORES, 2, P, 2, 4, QB).transpose(0, 4, 2, 1, 3, 5)
    )
    # y8 [N, 2pr, 128, 2, 4096] -> [N, 4quarter, 128, 2pr, 2i, 1024]
    y8 = np.ascontiguousarray(
        y8.reshape(N_CORES, 2, P, 2, 4, K // 4).transpose(0, 4, 2, 1, 3, 5)
    )
    x0h = np.ascontiguousarray(x8[:, 0, :, :, :, 0:P])
    return [
        {"x8d": x8[n], "y8d": y8[n], "A8d": A8d, "Wv8d": Wv8d, "x0h": x0h[n]}
        for n in range(N_CORES)
    ]


def run_device(x, y, Wq, Wk, Wv, Wo, **spmd_kwargs):
    nc = _get_nc()
    in_maps = _prep_inputs(x, y, Wq, Wk, Wv, Wo)
    res = run_bass_kernel_spmd(nc, in_maps, core_ids=list(range(N_CORES)), **spmd_kwargs)
    att = np.stack([res.results[n]["out"].astype(np.float32) for n in range(N_CORES)])
    return att, res


def kernel(x, y, Wq, Wk, Wv, Wo, bo):
    x = np.asarray(x, dtype=np.float32)
    y = np.asarray(y, dtype=np.float32)
    Wq = np.asarray(Wq, dtype=np.float32)
    Wk = np.asarray(Wk, dtype=np.float32)
    Wv = np.asarray(Wv, dtype=np.float32)
    Wo = np.asarray(Wo, dtype=np.float32)
    bo = np.asarray(bo, dtype=np.float32)
    att, _ = run_device(x, y, Wq, Wk, Wv, Wo)
    return x + att + bo[None, None, :]
